# revision 1
# baseline (speedup 1.0000x reference)
"""Trainium2 Bass/Tile kernel for nn_LrFeatureUpScaler (TransformerConv on a
fully-connected graph + GraphNorm + per-node L2 norm), SPMD over 8 NeuronCores.

Sharding: target nodes (rows i) are sharded 512/core. Each core computes its
own k/v row-shard (transposed-k / native-v), the shards are exchanged with one
AllGather, attention + skip run fully local, and GraphNorm per-channel stats
are combined with one small AllReduce.

Self-contained: hardcodes all shapes; no sibling imports.
"""

import sys

for _p in ("/opt/trn_rl_repo", "/opt/trn_rl_repo/concourse"):
    if _p not in sys.path:
        sys.path.insert(0, _p)

import numpy as np

import concourse.bass as bass
import concourse.tile as tile
from concourse import bacc, mybir
from concourse.bass_utils import run_bass_kernel_spmd
from concourse.masks import make_identity

N = 4096          # nodes == lr feature dim
H = 8             # heads
C = 512           # channels
D = C // H        # head dim = 64
M = 8             # cores
B = N // M        # rows per core = 512
PB = N // 128     # 32 p-blocks (contraction tiles)
CT = C // 128     # 4 channel tiles
JT = N // 128     # 32 j tiles
EPS = 1e-5

F32 = mybir.dt.float32
BF16 = mybir.dt.bfloat16
AF = mybir.ActivationFunctionType
ALU = mybir.AluOpType


def _emit_once(nc, tc, io, groups, no_cc):
    (x_rows, x_cols, Wq, Wk, Wv, Wskip, bq, bk, bv, bskip, we,
     gn_w, gn_b, gn_ms, out) = io
    with (
        tc.tile_pool(name="consts", bufs=1) as consts,
        tc.tile_pool(name="big", bufs=1) as big,
        tc.tile_pool(name="small", bufs=2) as small,
        tc.tile_pool(name="dram", bufs=1, space="DRAM") as dram,
    ):
        # ---------------- stage 0: constants ----------------
        ident_bf = consts.tile([128, 128], BF16)
        make_identity(nc, ident_bf)
        ident_f32 = consts.tile([128, 128], F32)
        make_identity(nc, ident_f32)

        # per-c-tile vectors: [128, CT] layout c = ct*128 + p
        def load_cvec(dram_t, dt=F32):
            t = consts.tile([128, CT], dt, name=f"cvec_{dram_t.name}")
            nc.gpsimd.dma_start(
                out=t, in_=dram_t.ap().rearrange("(t p) -> p t", p=128)
            )
            return t

        bq_sb = load_cvec(bq)
        bk_sb = load_cvec(bk)
        bskip_sb = load_cvec(bskip)
        gnw_sb = load_cvec(gn_w)
        gnb_sb = load_cvec(gn_b)
        gnms_sb = load_cvec(gn_ms)
        we_col = load_cvec(we)

        # bv broadcast across partitions: [128, C] f32
        bv_b = consts.tile([128, C], F32)
        nc.gpsimd.dma_start(
            out=bv_b, in_=bv.ap().unsqueeze(0).partition_broadcast(128)
        )

        # we_aug [128, H, D+1] bf16 (col D stays 0)
        we_aug = consts.tile([128, H, D + 1], BF16)
        nc.vector.memset(we_aug, 0.0)
        for h in range(H):
            nc.gpsimd.dma_start(
                out=we_aug[:, h, 0:D],
                in_=we.ap()[h * D:(h + 1) * D].unsqueeze(0).partition_broadcast(128),
            )

        ones_col = consts.tile([128, 1], F32)
        nc.vector.memset(ones_col, 1.0)
        eps_col = consts.tile([128, 1], F32)
        nc.vector.memset(eps_col, EPS)
        ones_row = consts.tile([1, 128], F32)
        nc.vector.memset(ones_row, 1.0)
        # we_rep[cp0+d, h, m] = we[h*D+d] for all m — stationary operand that
        # computes qe already broadcast across partitions
        ones_blk = consts.tile([128, 128], BF16)
        nc.vector.memset(ones_blk, 1.0)
        we_rep = consts.tile([128, H, 128], BF16)
        for h in range(H):
            cp0 = (h % 2) * D
            nc.vector.tensor_scalar(
                out=we_rep[cp0:cp0 + D, h, :],
                in0=ones_blk[cp0:cp0 + D, :],
                scalar1=we_col[cp0:cp0 + D, h // 2:h // 2 + 1],
                scalar2=None,
                op0=ALU.mult,
            )

        # ---------------- stage 1: x_rows -> bf16 -> transposed --------------
        x_bf_dram = dram.tile([B, N], BF16)
        for it in range(4):
            nc.gpsimd.dma_start(
                out=x_bf_dram[it * 128:(it + 1) * 128, :],
                in_=x_rows[it * 128:(it + 1) * 128, :],
            )

        # weights -> DRAM bf16 staging (few big SWDGE cast DMAs; tile loads
        # then go through HWDGE, keeping the Pool engine free)
        w_bf = {}
        for W_dram in (Wk, Wv, Wq, Wskip):
            wbf = dram.tile([N, C], BF16, name=f"wbf_{W_dram.name}")
            for q in range(4):
                nc.gpsimd.dma_start(
                    out=wbf[q * (N // 4):(q + 1) * (N // 4), :],
                    in_=W_dram[q * (N // 4):(q + 1) * (N // 4), :],
                )
            w_bf[W_dram.name] = wbf

        # xa staging: x_cols -> DRAM bf16, then HWDGE loads
        xa_bf_dram = dram.tile([N, B], BF16)
        for q in range(4):
            nc.gpsimd.dma_start(
                out=xa_bf_dram[q * (N // 4):(q + 1) * (N // 4), :],
                in_=x_cols[q * (N // 4):(q + 1) * (N // 4), :],
            )
        # xa [128, JT, B] bf16: xa[jp, jt, i] = x_cols[jt*128+jp, i]
        xa = big.tile([128, JT, B], BF16)
        for jt in range(JT):
            nc.sync.dma_start(
                out=xa[:, jt, :], in_=xa_bf_dram[jt * 128:(jt + 1) * 128, :]
            )

        cc_in = dram.tile([2, B, C], BF16)
        cc_out_k = dram.tile([M, B, C], BF16)
        cc_out_v = dram.tile([M, B, C], BF16)

        kT = big.tile([128, CT, N], BF16)     # kT[cp, ct, j]
        qT = big.tile([128, CT, B], BF16)     # qT[cp, ct, i]
        outT = big.tile([128, CT, B], F32)    # pre-norm out, transposed
        v_aug = big.tile([128, JT, H, D + 1], BF16)
        nc.vector.memset(v_aug[:, :, :, D:D + 1], 1.0)
        qe_b = consts.tile([128, H, B], BF16)

        with (
            tc.tile_pool(name="xtpool", bufs=1) as xtpool,
            tc.tile_pool(name="wstream", bufs=1) as wstream,
            tc.tile_pool(name="locstage", bufs=1) as locstage,
            tc.tile_pool(name="psum_proj", bufs=1, space="PSUM") as pp,
        ):
            # xT [128, PB, B] bf16: xT[p, pb, i] = x_rows[i, pb*128+p]
            xT = xtpool.tile([128, PB, B], BF16)
            for pb in range(PB):
                nc.sync.dma_start(
                    out=xT[:, pb, :],
                    in_=x_bf_dram[:, pb * 128:(pb + 1) * 128],
                    transpose=True,
                )

            # -------- stage 2a: kT_local + v_local, AllGather --------
            def projT(W_dram, bias_sb, dst_ap):
                """dst[cp, ct, i] = sum_p xT[p, :, i]*W[p, ct*128+cp] + bias."""
                wbf = w_bf[W_dram.name]
                psums = [
                    pp.tile([128, B], F32, tag=f"pp{ct}",
                            name=f"psum_{W_dram.name}_{ct}")
                    for ct in range(CT)
                ]
                for pb in range(PB):
                    wt = wstream.tile([128, C], BF16, tag="wt", bufs=4,
                                      name=f"wt_{W_dram.name}")
                    nc.sync.dma_start(
                        out=wt, in_=wbf[pb * 128:(pb + 1) * 128, :]
                    )
                    for ct in range(CT):
                        nc.tensor.matmul(
                            psums[ct],
                            lhsT=wt[:, ct * 128:(ct + 1) * 128],
                            rhs=xT[:, pb, :],
                            start=(pb == 0),
                            stop=(pb == PB - 1),
                        )
                for ct in range(CT):
                    nc.vector.tensor_scalar(
                        out=dst_ap[:, ct, :],
                        in0=psums[ct],
                        scalar1=bias_sb[:, ct:ct + 1],
                        scalar2=None,
                        op0=ALU.add,
                    )

            kT_loc = locstage.tile([128, CT, B], BF16)
            projT(Wk, bk_sb, kT_loc)
            nc.sync.dma_start(
                out=cc_in[0].rearrange("(ct cp) j -> cp ct j", cp=128),
                in_=kT_loc,
            )
            if no_cc:
                for r in range(M):
                    nc.sync.dma_start(out=cc_out_k[r], in_=cc_in[0])
            else:
                nc.gpsimd.collective_compute(
                    "AllGather",
                    ALU.bypass,
                    replica_groups=groups,
                    ins=[cc_in[0].opt()],
                    outs=[cc_out_k.opt()],
                )

            # v_local native: v[jp, jtl, c] = sum_p x[jtl*128+jp, p]*Wv[p, c]
            v_loc = locstage.tile([128, 4, C], BF16)
            for jtl in range(4):
                psv = pp.tile([128, C], F32, tag="pp0", name=f"psum_v_{jtl}")
                for pb in range(PB):
                    wt = wstream.tile([128, C], BF16, tag="wt", bufs=4,
                                      name="wt_v")
                    nc.sync.dma_start(
                        out=wt, in_=w_bf["Wv"][pb * 128:(pb + 1) * 128, :]
                    )
                    nc.tensor.matmul(
                        psv,
                        lhsT=xT[:, pb, jtl * 128:(jtl + 1) * 128],
                        rhs=wt,
                        start=(pb == 0),
                        stop=(pb == PB - 1),
                    )
                nc.vector.tensor_tensor(
                    out=v_loc[:, jtl, :], in0=psv, in1=bv_b, op=ALU.add
                )
            nc.sync.dma_start(
                out=cc_in[1].rearrange("(jtl jp) c -> jp jtl c", jp=128),
                in_=v_loc,
            )
            if no_cc:
                for r in range(M):
                    nc.sync.dma_start(out=cc_out_v[r], in_=cc_in[1])
            else:
                nc.gpsimd.collective_compute(
                    "AllGather",
                    ALU.bypass,
                    replica_groups=groups,
                    ins=[cc_in[1].opt()],
                    outs=[cc_out_v.opt()],
                )

            # -------- stage 2b: qT, skipT, qe --------
            projT(Wq, bq_sb, qT)
            projT(Wskip, bskip_sb, outT)

            # qe_b[:, h, i] = sum_d we[h*D+d]*qT[h*D+d, i]  (rows identical)
            for h in range(H):
                cp0 = (h % 2) * D
                ct = h // 2
                pq = pp.tile([128, B], F32, tag="pq", bufs=2, name=f"psum_qe_{h}")
                nc.tensor.matmul(
                    pq,
                    lhsT=we_rep[cp0:cp0 + D, h, :],
                    rhs=qT[cp0:cp0 + D, ct, :],
                )
                nc.vector.tensor_copy(out=qe_b[:, h, :], in_=pq)

        # ---------------- stage 3: unpack gathered kT / v ----------------
        for r in range(M):
            for ct in range(CT):
                nc.sync.dma_start(
                    out=kT[:, ct, r * B:(r + 1) * B],
                    in_=cc_out_k[r, ct * 128:(ct + 1) * 128, :],
                )
        for jt in range(JT):
            r, jtl = jt // 4, jt % 4
            nc.sync.dma_start(
                out=v_aug[:, jt, :, 0:D],
                in_=cc_out_v[r, jtl * 128:(jtl + 1) * 128, :].rearrange(
                    "p (h d) -> p h d", h=H
                ),
            )

        # ---------------- stage 4: attention ----------------
        with (
            tc.tile_pool(name="psum_att", bufs=1, space="PSUM") as pa,
            tc.tile_pool(name="att", bufs=1) as att,
        ):
            for h in range(H):
                cp0 = (h % 2) * D
                ct = h // 2
                po = pa.tile([D + 1, B], F32, tag="po", bufs=2, name=f"po_{h}")
                for jt in range(JT):
                    ps = pa.tile([128, B], F32, tag="ps", bufs=3,
                                 name=f"ps_{h}_{jt}")
                    nc.tensor.matmul(
                        ps,
                        lhsT=kT[cp0:cp0 + D, ct, jt * 128:(jt + 1) * 128],
                        rhs=qT[cp0:cp0 + D, ct, :],
                        start=True,
                        stop=False,
                    )
                    tmp = att.tile([128, B], BF16, tag="tmp", bufs=4,
                                   name=f"tmp_{h}_{jt}")
                    nc.vector.tensor_tensor(
                        out=tmp, in0=xa[:, jt, :], in1=qe_b[:, h, :], op=ALU.mult
                    )
                    nc.tensor.matmul(
                        ps, lhsT=ident_bf, rhs=tmp, start=False, stop=True
                    )
                    alpha = att.tile([128, B], BF16, tag="alpha", bufs=4,
                                     name=f"alpha_{h}_{jt}")
                    nc.scalar.activation(
                        out=alpha, in_=ps, func=AF.Exp, scale=0.125
                    )
                    mt = att.tile([128, B], BF16, tag="mt", bufs=4,
                                  name=f"mt_{h}_{jt}")
                    nc.vector.tensor_tensor(
                        out=mt, in0=alpha, in1=xa[:, jt, :], op=ALU.mult
                    )
                    nc.tensor.matmul(
                        po,
                        lhsT=v_aug[:, jt, h, :],
                        rhs=alpha,
                        start=(jt == 0),
                        stop=False,
                        skip_group_check=True,
                    )
                    nc.tensor.matmul(
                        po,
                        lhsT=we_aug[:, h, :],
                        rhs=mt,
                        start=False,
                        stop=(jt == JT - 1),
                        skip_group_check=True,
                    )
                # epilogue: outT[h rows] += po[0:D] / po[D]
                rz = small.tile([1, B], F32, tag="rz", name=f"rz_{h}")
                nc.vector.reciprocal(out=rz, in_=po[D:D + 1, :])
                prz = pa.tile([D, B], F32, tag="prz", bufs=2, name=f"prz_{h}")
                nc.tensor.matmul(prz, lhsT=ones_row[:, 0:D], rhs=rz)
                rz_bf = small.tile([128, B], F32, tag="rz_b", name=f"rz_b_{h}")
                rz_b = rz_bf[cp0:cp0 + D, :]
                nc.vector.tensor_copy(out=rz_b, in_=prz)
                t1f = small.tile([128, B], F32, tag="t1", name=f"t1_{h}")
                t1 = t1f[cp0:cp0 + D, :]
                nc.vector.tensor_tensor(
                    out=t1, in0=po[0:D, :], in1=rz_b, op=ALU.mult
                )
                nc.vector.tensor_tensor(
                    out=outT[cp0:cp0 + D, ct, :],
                    in0=outT[cp0:cp0 + D, ct, :],
                    in1=t1,
                    op=ALU.add,
                )

        # ---------------- stage 5: GraphNorm + L2 + emit ----------------
        with (
            tc.tile_pool(name="fin", bufs=1) as fin,
            tc.tile_pool(name="psum_f", bufs=1, space="PSUM") as pf,
        ):
            st_in = dram.tile([128, 2 * CT], F32)
            st_out = dram.tile([128, 2 * CT], F32)
            stats = small.tile([128, 2 * CT], F32, bufs=1)
            scr = fin.tile([128, B], F32)
            for ct in range(CT):
                sm = small.tile([128, 1], F32, tag="sm", name=f"sm_{ct}")
                nc.vector.tensor_reduce(
                    out=sm, in_=outT[:, ct, :], axis=mybir.AxisListType.X,
                    op=ALU.add,
                )
                ss = small.tile([128, 1], F32, tag="ss", name=f"ss_{ct}")
                nc.scalar.activation(out=scr, in_=outT[:, ct, :], func=AF.Square)
                nc.vector.tensor_reduce(
                    out=ss, in_=scr, axis=mybir.AxisListType.X, op=ALU.add
                )
                nc.vector.tensor_scalar(
                    out=stats[:, 2 * ct:2 * ct + 1], in0=sm,
                    scalar1=1.0 / N, scalar2=None, op0=ALU.mult,
                )
                nc.vector.tensor_scalar(
                    out=stats[:, 2 * ct + 1:2 * ct + 2], in0=ss,
                    scalar1=1.0 / N, scalar2=None, op0=ALU.mult,
                )
            nc.sync.dma_start(out=st_in, in_=stats)
            if no_cc:
                nc.sync.dma_start(out=st_out, in_=st_in)
            else:
                nc.gpsimd.collective_compute(
                    "AllReduce",
                    ALU.add,
                    replica_groups=groups,
                    ins=[st_in.opt()],
                    outs=[st_out.opt()],
                )
            gstats = small.tile([128, 2 * CT], F32, bufs=1)
            nc.sync.dma_start(out=gstats, in_=st_out)

            pl2 = pf.tile([1, B], F32, tag="pl2", bufs=1)
            for ct in range(CT):
                EX = gstats[:, 2 * ct:2 * ct + 1]
                EX2 = gstats[:, 2 * ct + 1:2 * ct + 2]
                msv = gnms_sb[:, ct:ct + 1]
                t2 = small.tile([128, 1], F32, tag="n_t", name=f"nt_{ct}")
                nc.vector.tensor_tensor(out=t2, in0=EX, in1=EX, op=ALU.mult)
                w1 = small.tile([128, 1], F32, tag="n_w", name=f"nw_{ct}")
                nc.vector.tensor_scalar(
                    out=w1, in0=msv, scalar1=-1.0, scalar2=2.0,
                    op0=ALU.mult, op1=ALU.add,
                )  # 2 - ms
                nc.vector.tensor_tensor(out=w1, in0=msv, in1=w1, op=ALU.mult)
                nc.vector.tensor_tensor(out=t2, in0=t2, in1=w1, op=ALU.mult)
                var = small.tile([128, 1], F32, tag="n_var", name=f"nvar_{ct}")
                nc.vector.tensor_tensor(out=var, in0=EX2, in1=t2, op=ALU.subtract)
                sd = small.tile([128, 1], F32, tag="n_sd", name=f"nsd_{ct}")
                nc.scalar.activation(out=sd, in_=var, func=AF.Sqrt, bias=eps_col)
                rstd = small.tile([128, 1], F32, tag="n_rstd", name=f"nrstd_{ct}")
                nc.vector.reciprocal(out=rstd, in_=sd)
                A = small.tile([128, 1], F32, tag="n_A", name=f"nA_{ct}")
                nc.vector.tensor_tensor(
                    out=A, in0=gnw_sb[:, ct:ct + 1], in1=rstd, op=ALU.mult
                )
                p1 = small.tile([128, 1], F32, tag="n_p1", name=f"np1_{ct}")
                nc.vector.tensor_tensor(out=p1, in0=A, in1=msv, op=ALU.mult)
                nc.vector.tensor_tensor(out=p1, in0=p1, in1=EX, op=ALU.mult)
                Bc = small.tile([128, 1], F32, tag="n_B", name=f"nB_{ct}")
                nc.vector.tensor_tensor(
                    out=Bc, in0=gnb_sb[:, ct:ct + 1], in1=p1, op=ALU.subtract
                )
                nc.vector.tensor_scalar(
                    out=outT[:, ct, :], in0=outT[:, ct, :],
                    scalar1=A, scalar2=Bc, op0=ALU.mult, op1=ALU.add,
                )
                sq = fin.tile([128, B], F32, tag="sq", bufs=2, name=f"sq_{ct}")
                nc.scalar.activation(out=sq, in_=outT[:, ct, :], func=AF.Square)
                nc.tensor.matmul(
                    pl2, lhsT=ones_col, rhs=sq,
                    start=(ct == 0), stop=(ct == CT - 1),
                )
            rn = small.tile([1, B], F32, bufs=1)
            nc.scalar.activation(out=rn, in_=pl2, func=AF.Sqrt)
            nc.vector.reciprocal(out=rn, in_=rn)
            prn = pf.tile([128, B], F32, tag="prn", bufs=1)
            nc.tensor.matmul(prn, lhsT=ones_row, rhs=rn)
            rn_b = fin.tile([128, B], F32)
            nc.vector.tensor_copy(out=rn_b, in_=prn)

            final = fin.tile([128, 4, C], F32)  # [ip, it, c]
            for ct in range(CT):
                nc.vector.tensor_tensor(
                    out=outT[:, ct, :], in0=outT[:, ct, :], in1=rn_b, op=ALU.mult
                )
                for it in range(4):
                    pt = pf.tile([128, 128], F32, tag="pt", bufs=4,
                                 name=f"pt_{ct}_{it}")
                    nc.tensor.transpose(
                        pt, outT[:, ct, it * 128:(it + 1) * 128], ident_f32
                    )
                    nc.vector.tensor_copy(
                        out=final[:, it, ct * 128:(ct + 1) * 128], in_=pt
                    )
            for it in range(4):
                nc.sync.dma_start(
                    out=out[it * 128:(it + 1) * 128, :], in_=final[:, it, :]
                )


def build_kernel(no_cc=False, n_cores=M, repeat=1):
    nc = bacc.Bacc("TRN2", target_bir_lowering=False, debug=False,
                   num_devices=n_cores)

    x_rows = nc.dram_tensor("x_rows", [B, N], F32, kind="ExternalInput")
    x_cols = nc.dram_tensor("x_cols", [N, B], F32, kind="ExternalInput")
    Wq = nc.dram_tensor("Wq", [N, C], F32, kind="ExternalInput")
    Wk = nc.dram_tensor("Wk", [N, C], F32, kind="ExternalInput")
    Wv = nc.dram_tensor("Wv", [N, C], F32, kind="ExternalInput")
    Wskip = nc.dram_tensor("Wskip", [N, C], F32, kind="ExternalInput")
    bq = nc.dram_tensor("bq", [C], F32, kind="ExternalInput")
    bk = nc.dram_tensor("bk", [C], F32, kind="ExternalInput")
    bv = nc.dram_tensor("bv", [C], F32, kind="ExternalInput")
    bskip = nc.dram_tensor("bskip", [C], F32, kind="ExternalInput")
    we = nc.dram_tensor("we", [C], F32, kind="ExternalInput")
    gn_w = nc.dram_tensor("gn_w", [C], F32, kind="ExternalInput")
    gn_b = nc.dram_tensor("gn_b", [C], F32, kind="ExternalInput")
    gn_ms = nc.dram_tensor("gn_ms", [C], F32, kind="ExternalInput")
    out = nc.dram_tensor("out", [B, C], F32, kind="ExternalOutput")

    io = (x_rows, x_cols, Wq, Wk, Wv, Wskip, bq, bk, bv, bskip, we,
          gn_w, gn_b, gn_ms, out)
    groups = [list(range(n_cores))]

    with tile.TileContext(nc) as tc:
        for _rep in range(repeat):
            _emit_once(nc, tc, io, groups, no_cc)

    nc.finalize()
    return nc


_NC_CACHE = {}


def kernel(**inputs):
    x = np.ascontiguousarray(inputs["x"], dtype=np.float32)
    if "nc" not in _NC_CACHE:
        _NC_CACHE["nc"] = build_kernel()
    nc = _NC_CACHE["nc"]

    in_maps = []
    for m in range(M):
        I = slice(m * B, (m + 1) * B)
        im = {
            "x_rows": np.ascontiguousarray(x[I, :]),
            "x_cols": np.ascontiguousarray(x[:, I]),
        }
        for k in ("Wq", "Wk", "Wv", "Wskip", "bq", "bk", "bv", "bskip",
                  "we", "gn_w", "gn_b", "gn_ms"):
            im[k] = np.ascontiguousarray(inputs[k], dtype=np.float32)
        in_maps.append(im)

    res = run_bass_kernel_spmd(nc, in_maps, core_ids=list(range(M)))
    return np.concatenate([res.results[m]["out"] for m in range(M)], axis=0)


if __name__ == "__main__":
    data = np.load("/tmp/inputs.npz")
    out = kernel(**{k: data[k] for k in data.files})
    ref = np.load("/tmp/ref_out.npy")
    err = np.abs(out - ref)
    print("absmax", err.max(), "scale-rel", err.max() / np.abs(ref).max())
    print("rel2", np.linalg.norm(out - ref) / np.linalg.norm(ref))



# revision 12
# speedup vs baseline: 1.3067x; 1.3067x over previous
"""Trainium2 Bass/Tile kernel for nn_LrFeatureUpScaler (TransformerConv on a
fully-connected graph + GraphNorm + per-node L2 norm), SPMD over 8 NeuronCores.

Sharding: target nodes (rows i) are sharded 512/core. Each core computes its
own k/v row-shard, the shards are exchanged with two AllGathers (overlapped
with the q/skip projections), attention + skip run fully local, and GraphNorm
per-channel stats are combined with one small AllReduce.

Layout strategy: all large inputs are pre-cast to bf16 and pre-transposed on
the host (x.T column-shard, x column-shard, bf16 weights), so the device loads
them straight into SBUF with no staging casts/transposes. Attention uses two
matmuls per (head, j-tile) — qk and alpha@v — with the edge-attr score term
fused into one DVE scalar_tensor_tensor, the xa*qe product on the Pool engine,
and the s = sum(alpha*xa) accumulation split between PE and DVE.

Self-contained: hardcodes all shapes; no sibling imports.
"""

import sys

for _p in ("/opt/trn_rl_repo", "/opt/trn_rl_repo/concourse"):
    if _p not in sys.path:
        sys.path.insert(0, _p)

import numpy as np
import ml_dtypes

import concourse.bass as bass
import concourse.tile as tile
from concourse import bacc, mybir
from concourse.bass_utils import run_bass_kernel_spmd
from concourse.masks import make_identity

N = 4096          # nodes == lr feature dim
H = 8             # heads
C = 512           # channels
D = C // H        # head dim = 64
M = 8             # cores
B = N // M        # rows per core = 512
PB = N // 128     # 32 p-blocks (contraction tiles)
CT = C // 128     # 4 channel tiles
JT = N // 128     # 32 j tiles
EPS = 1e-5
PE_FOLD_EVERY = 3  # 1 of every 3 j-tiles folds s on PE; rest accumulate on DVE

F32 = mybir.dt.float32
BF16 = mybir.dt.bfloat16
AF = mybir.ActivationFunctionType
ALU = mybir.AluOpType
BF16_NP = ml_dtypes.bfloat16


def _emit_once(nc, tc, io, groups, no_cc, stage=3):
    (xT_in, xa_in, Wq, Wk, Wv, Wskip, bq, bk, bv, bskip, we,
     gn_w, gn_b, gn_ms, out) = io
    with (
        tc.tile_pool(name="consts", bufs=1) as consts,
        tc.tile_pool(name="keep", bufs=1) as keep,
        tc.tile_pool(name="small", bufs=2) as small,
        tc.tile_pool(name="dram", bufs=1, space="DRAM") as dram,
    ):
        # ---------------- constants ----------------
        ident_f32 = consts.tile([128, 128], F32)
        make_identity(nc, ident_f32)

        # per-c-tile vectors: [128, CT] layout c = ct*128 + p
        def load_cvec(dram_t, dt=F32):
            t = consts.tile([128, CT], dt, name=f"cvec_{dram_t.name}")
            nc.gpsimd.dma_start(
                out=t, in_=dram_t.ap().rearrange("(t p) -> p t", p=128)
            )
            return t

        bq_sb = load_cvec(bq)
        bk_sb = load_cvec(bk)
        bskip_sb = load_cvec(bskip)
        gnw_sb = load_cvec(gn_w)
        gnb_sb = load_cvec(gn_b)
        gnms_sb = load_cvec(gn_ms)
        we_col = load_cvec(we)

        # bv broadcast across partitions: [128, C] f32
        bv_b = consts.tile([128, C], F32)
        nc.gpsimd.dma_start(
            out=bv_b, in_=bv.ap().unsqueeze(0).partition_broadcast(128)
        )

        # we_aug [128, H, D+1] (col D stays 0): bf16 for per-tile PE s-folds,
        # f32 for the end-of-head fold of the DVE accumulator.
        we_aug_bf = consts.tile([128, H, D + 1], BF16)
        nc.vector.memset(we_aug_bf, 0.0)
        we_aug_f = consts.tile([128, H, D + 1], F32)
        nc.vector.memset(we_aug_f, 0.0)
        for h in range(H):
            nc.gpsimd.dma_start(
                out=we_aug_bf[:, h, 0:D],
                in_=we.ap()[h * D:(h + 1) * D].unsqueeze(0).partition_broadcast(128),
            )
            nc.gpsimd.dma_start(
                out=we_aug_f[:, h, 0:D],
                in_=we.ap()[h * D:(h + 1) * D].unsqueeze(0).partition_broadcast(128),
            )

        ones_col = consts.tile([128, 1], F32)
        nc.vector.memset(ones_col, 1.0)
        eps_col = consts.tile([128, 1], F32)
        nc.vector.memset(eps_col, EPS)
        ones_row = consts.tile([1, 128], F32)
        nc.vector.memset(ones_row, 1.0)
        # we_rep8[cp0+d, h, m] = 0.125*we[h*D+d] for all m — stationary operand
        # that computes qe pre-scaled by 1/sqrt(D) and broadcast across
        # partitions.
        ones8_blk = consts.tile([128, 128], BF16)
        nc.vector.memset(ones8_blk, 0.125)
        we_rep8 = consts.tile([128, H, 128], BF16)
        for h in range(H):
            cp0 = (h % 2) * D
            nc.vector.tensor_scalar(
                out=we_rep8[cp0:cp0 + D, h, :],
                in0=ones8_blk[cp0:cp0 + D, :],
                scalar1=we_col[cp0:cp0 + D, h // 2:h // 2 + 1],
                scalar2=None,
                op0=ALU.mult,
            )

        # ---------------- bulk input loads (no staging) ----------------
        cc_in_k = dram.tile([B, C], BF16)
        cc_in_v = dram.tile([B, H * (D + 1)], BF16)
        cc_out_k = dram.tile([M, B, C], BF16)
        cc_out_v = dram.tile([M, B, H * (D + 1)], BF16)

        kT = keep.tile([128, CT, N], BF16)     # kT[cp, ct, j]
        qT = keep.tile([128, CT, B], BF16)     # qT[cp, ct, i]
        outT = keep.tile([128, CT, B], F32)    # pre-norm out, transposed
        xa = keep.tile([128, JT, B], BF16)     # xa[jp, jt, i] = x[j_glob, i_glob]
        v_aug = keep.tile([128, JT, H, D + 1], BF16)
        qe8_b = keep.tile([128, H, B], BF16)

        with (
            tc.tile_pool(name="xtpool", bufs=1) as xtpool,
            tc.tile_pool(name="wstream", bufs=1) as wstream,
            tc.tile_pool(name="locstage", bufs=1) as locstage,
            tc.tile_pool(name="psum_proj", bufs=1, space="PSUM") as pp,
        ):
            # xT [128, PB, B] bf16: xT[p, pb, i] = x[i_global, pb*128+p]
            xT = xtpool.tile([128, PB, B], BF16)
            for pb in range(PB):
                nc.sync.dma_start(
                    out=xT[:, pb, :], in_=xT_in[pb * 128:(pb + 1) * 128, :]
                )
            # xa loads (needed only at attention time) follow the xT loads
            for jt in range(JT):
                nc.sync.dma_start(
                    out=xa[:, jt, :], in_=xa_in[jt * 128:(jt + 1) * 128, :]
                )

            def projT(W_dram, bias_sb, dst_ap):
                """dst[cp, ct, i] = sum_p xT[p, :, i]*W[p, ct*128+cp] + bias."""
                psums = [
                    pp.tile([128, B], F32, tag=f"pp{ct}",
                            name=f"psum_{W_dram.name}_{ct}")
                    for ct in range(CT)
                ]
                for pb in range(PB):
                    wt = wstream.tile([128, C], BF16, tag="wt", bufs=4,
                                      name=f"wt_{W_dram.name}")
                    nc.scalar.dma_start(
                        out=wt, in_=W_dram[pb * 128:(pb + 1) * 128, :]
                    )
                    for ct in range(CT):
                        nc.tensor.matmul(
                            psums[ct],
                            lhsT=wt[:, ct * 128:(ct + 1) * 128],
                            rhs=xT[:, pb, :],
                            start=(pb == 0),
                            stop=(pb == PB - 1),
                        )
                for ct in range(CT):
                    nc.vector.tensor_scalar(
                        out=dst_ap[:, ct, :],
                        in0=psums[ct],
                        scalar1=bias_sb[:, ct:ct + 1],
                        scalar2=None,
                        op0=ALU.add,
                    )

            # -------- k shard + AllGather (fired early) --------
            kT_loc = locstage.tile([128, CT, B], BF16)
            projT(Wk, bk_sb, kT_loc)
            nc.sync.dma_start(
                out=cc_in_k.rearrange("(ct cp) j -> cp ct j", cp=128),
                in_=kT_loc,
            )
            if no_cc:
                for r in range(M):
                    nc.sync.dma_start(out=cc_out_k[r], in_=cc_in_k)
            else:
                nc.gpsimd.collective_compute(
                    "AllGather",
                    ALU.bypass,
                    replica_groups=groups,
                    ins=[cc_in_k.opt()],
                    outs=[cc_out_k.opt()],
                )

            # -------- v shard (augmented with the Z ones-column) + AllGather --
            # v[jp, jtl, c] = sum_p x[jtl*128+jp, p]*Wv[p, c]; Wv read once.
            v_loc = locstage.tile([128, 4, H, D + 1], BF16)
            nc.vector.memset(v_loc[:, :, :, D:D + 1], 1.0)
            psvs = [
                pp.tile([128, C], F32, tag=f"pp{jtl}", name=f"psum_v_{jtl}")
                for jtl in range(4)
            ]
            for pb in range(PB):
                wt = wstream.tile([128, C], BF16, tag="wt", bufs=4, name="wt_v")
                nc.scalar.dma_start(
                    out=wt, in_=Wv[pb * 128:(pb + 1) * 128, :]
                )
                for jtl in range(4):
                    nc.tensor.matmul(
                        psvs[jtl],
                        lhsT=xT[:, pb, jtl * 128:(jtl + 1) * 128],
                        rhs=wt,
                        start=(pb == 0),
                        stop=(pb == PB - 1),
                    )
            for jtl in range(4):
                nc.vector.tensor_tensor(
                    out=v_loc[:, jtl, :, 0:D],
                    in0=psvs[jtl].rearrange("p (h d) -> p h d", h=H),
                    in1=bv_b.rearrange("p (h d) -> p h d", h=H),
                    op=ALU.add,
                )
            nc.sync.dma_start(
                out=cc_in_v.rearrange("(jtl jp) f -> jp jtl f", jp=128),
                in_=v_loc,
            )
            if no_cc:
                for r in range(M):
                    nc.sync.dma_start(out=cc_out_v[r], in_=cc_in_v)
            else:
                nc.gpsimd.collective_compute(
                    "AllGather",
                    ALU.bypass,
                    replica_groups=groups,
                    ins=[cc_in_v.opt()],
                    outs=[cc_out_v.opt()],
                )

            # -------- q, skip, qe (overlap the AllGathers) --------
            projT(Wq, bq_sb, qT)
            projT(Wskip, bskip_sb, outT)

            # qe8_b[:, h, i] = 0.125 * sum_d we[h*D+d]*qT[h*D+d, i]
            for h in range(H):
                cp0 = (h % 2) * D
                ct = h // 2
                pq = pp.tile([128, B], F32, tag="pq", bufs=2, name=f"psum_qe_{h}")
                nc.tensor.matmul(
                    pq,
                    lhsT=we_rep8[cp0:cp0 + D, h, :],
                    rhs=qT[cp0:cp0 + D, ct, :],
                )
                nc.vector.tensor_copy(out=qe8_b[:, h, :], in_=pq)

        if stage <= 1:
            for ct in range(CT):
                nc.sync.dma_start(
                    out=out[ct * 128:(ct + 1) * 128, :], in_=outT[:, ct, :]
                )
            return

        # ---------------- unpack gathered kT / v (contiguous HWDGE loads) ----
        for r in range(M):
            for ct in range(CT):
                nc.sync.dma_start(
                    out=kT[:, ct, r * B:(r + 1) * B],
                    in_=cc_out_k[r, ct * 128:(ct + 1) * 128, :],
                )
        for jt in range(JT):
            r, jtl = jt // 4, jt % 4
            nc.scalar.dma_start(
                out=v_aug[:, jt, :, :],
                in_=cc_out_v[r, jtl * 128:(jtl + 1) * 128, :].rearrange(
                    "p (h d) -> p h d", h=H, d=D + 1
                ),
            )

        # ---------------- attention ----------------
        with (
            tc.tile_pool(name="psum_att", bufs=1, space="PSUM") as pa,
            tc.tile_pool(name="att", bufs=1) as att,
        ):
            acc_s = att.tile([128, B], F32, bufs=2)
            for h in range(H):
                cp0 = (h % 2) * D
                ct = h // 2
                po = pa.tile([D + 1, B], F32, tag="po", bufs=2, name=f"po_{h}")
                first_acc = True
                for jt in range(JT):
                    ps = pa.tile([128, B], F32, tag="ps", bufs=3,
                                 name=f"ps_{h}_{jt}")
                    nc.tensor.matmul(
                        ps,
                        lhsT=kT[cp0:cp0 + D, ct, jt * 128:(jt + 1) * 128],
                        rhs=qT[cp0:cp0 + D, ct, :],
                        start=True,
                        stop=True,
                    )
                    tmp = att.tile([128, B], BF16, tag="tmp", bufs=4,
                                   name=f"tmp_{h}_{jt}")
                    nc.vector.tensor_tensor(
                        out=tmp, in0=xa[:, jt, :], in1=qe8_b[:, h, :],
                        op=ALU.mult,
                    )
                    ps2 = att.tile([128, B], BF16, tag="ps2", bufs=4,
                                   name=f"ps2_{h}_{jt}")
                    nc.vector.scalar_tensor_tensor(
                        out=ps2, in0=ps, scalar=0.125, in1=tmp,
                        op0=ALU.mult, op1=ALU.add,
                    )
                    alpha = att.tile([128, B], BF16, tag="alpha", bufs=4,
                                     name=f"alpha_{h}_{jt}")
                    nc.scalar.activation(
                        out=alpha, in_=ps2, func=AF.Exp, scale=1.0
                    )
                    nc.tensor.matmul(
                        po,
                        lhsT=v_aug[:, jt, h, :],
                        rhs=alpha,
                        start=(jt == 0),
                        stop=False,
                        skip_group_check=True,
                    )
                    mt = att.tile([128, B], BF16, tag="mt", bufs=4,
                                  name=f"mt_{h}_{jt}")
                    nc.vector.tensor_tensor(
                        out=mt, in0=alpha, in1=xa[:, jt, :], op=ALU.mult
                    )
                    if jt % PE_FOLD_EVERY == 0:
                        nc.tensor.matmul(
                            po,
                            lhsT=we_aug_bf[:, h, :],
                            rhs=mt,
                            start=False,
                            stop=False,
                            skip_group_check=True,
                        )
                    elif first_acc:
                        nc.vector.tensor_copy(out=acc_s, in_=mt)
                        first_acc = False
                    else:
                        nc.vector.tensor_tensor(
                            out=acc_s, in0=acc_s, in1=mt, op=ALU.add
                        )
                # fold the DVE-side s accumulator, closing the po group
                nc.tensor.matmul(
                    po,
                    lhsT=we_aug_f[:, h, :],
                    rhs=acc_s,
                    start=False,
                    stop=True,
                    skip_group_check=True,
                )
                # epilogue: outT[h rows] += po[0:D] / po[D]
                rz = small.tile([1, B], F32, tag="rz", name=f"rz_{h}")
                nc.vector.reciprocal(out=rz, in_=po[D:D + 1, :])
                prz = pa.tile([D, B], F32, tag="prz", bufs=2, name=f"prz_{h}")
                nc.tensor.matmul(prz, lhsT=ones_row[:, 0:D], rhs=rz)
                rz_bf = small.tile([128, B], F32, tag="rz_b", name=f"rz_b_{h}")
                rz_b = rz_bf[cp0:cp0 + D, :]
                nc.vector.tensor_copy(out=rz_b, in_=prz)
                t1f = small.tile([128, B], F32, tag="t1", name=f"t1_{h}")
                t1 = t1f[cp0:cp0 + D, :]
                nc.vector.tensor_tensor(
                    out=t1, in0=po[0:D, :], in1=rz_b, op=ALU.mult
                )
                nc.vector.tensor_tensor(
                    out=outT[cp0:cp0 + D, ct, :],
                    in0=outT[cp0:cp0 + D, ct, :],
                    in1=t1,
                    op=ALU.add,
                )

        if stage <= 2:
            for ct in range(CT):
                nc.sync.dma_start(
                    out=out[ct * 128:(ct + 1) * 128, :], in_=outT[:, ct, :]
                )
            return

        # ---------------- GraphNorm + L2 + emit ----------------
        with (
            tc.tile_pool(name="fin", bufs=1) as fin,
            tc.tile_pool(name="psum_f", bufs=1, space="PSUM") as pf,
        ):
            st_in = dram.tile([128, 2 * CT], F32)
            st_out = dram.tile([128, 2 * CT], F32)
            stats = small.tile([128, 2 * CT], F32, bufs=1)
            scr = fin.tile([128, B], F32)
            for ct in range(CT):
                sm = small.tile([128, 1], F32, tag="sm", name=f"sm_{ct}")
                nc.vector.tensor_reduce(
                    out=sm, in_=outT[:, ct, :], axis=mybir.AxisListType.X,
                    op=ALU.add,
                )
                nc.vector.tensor_scalar(
                    out=stats[:, 2 * ct:2 * ct + 1], in0=sm,
                    scalar1=1.0 / N, scalar2=None, op0=ALU.mult,
                )
                nc.scalar.activation(out=scr, in_=outT[:, ct, :], func=AF.Square)
                ss = small.tile([128, 1], F32, tag="ss", name=f"ss_{ct}")
                nc.vector.tensor_reduce(
                    out=ss, in_=scr, axis=mybir.AxisListType.X, op=ALU.add
                )
                nc.vector.tensor_scalar(
                    out=stats[:, 2 * ct + 1:2 * ct + 2], in0=ss,
                    scalar1=1.0 / N, scalar2=None, op0=ALU.mult,
                )
            nc.sync.dma_start(out=st_in, in_=stats)
            if no_cc:
                nc.sync.dma_start(out=st_out, in_=st_in)
            else:
                nc.gpsimd.collective_compute(
                    "AllReduce",
                    ALU.add,
                    replica_groups=groups,
                    ins=[st_in.opt()],
                    outs=[st_out.opt()],
                )

            # transpose raw outT while the AllReduce runs
            pre = fin.tile([128, 4, C], F32)  # [ip, it, c]
            for ct in range(CT):
                for it in range(4):
                    pt = pf.tile([128, 128], F32, tag="pt", bufs=4,
                                 name=f"pt_{ct}_{it}")
                    nc.tensor.transpose(
                        pt, outT[:, ct, it * 128:(it + 1) * 128], ident_f32
                    )
                    nc.vector.tensor_copy(
                        out=pre[:, it, ct * 128:(ct + 1) * 128], in_=pt
                    )

            gstats = small.tile([128, 2 * CT], F32, bufs=1)
            nc.sync.dma_start(out=gstats, in_=st_out)

            # per-channel A (scale) and Bc (shift), in cvec layout
            A_cvec = small.tile([128, CT], F32, bufs=1)
            B_cvec = small.tile([128, CT], F32, bufs=1)
            for ct in range(CT):
                EX = gstats[:, 2 * ct:2 * ct + 1]
                EX2 = gstats[:, 2 * ct + 1:2 * ct + 2]
                msv = gnms_sb[:, ct:ct + 1]
                t2 = small.tile([128, 1], F32, tag="n_t", name=f"nt_{ct}")
                nc.vector.tensor_tensor(out=t2, in0=EX, in1=EX, op=ALU.mult)
                w1 = small.tile([128, 1], F32, tag="n_w", name=f"nw_{ct}")
                nc.vector.tensor_scalar(
                    out=w1, in0=msv, scalar1=-1.0, scalar2=2.0,
                    op0=ALU.mult, op1=ALU.add,
                )  # 2 - ms
                nc.vector.tensor_tensor(out=w1, in0=msv, in1=w1, op=ALU.mult)
                nc.vector.tensor_tensor(out=t2, in0=t2, in1=w1, op=ALU.mult)
                var = small.tile([128, 1], F32, tag="n_var", name=f"nvar_{ct}")
                nc.vector.tensor_tensor(out=var, in0=EX2, in1=t2, op=ALU.subtract)
                sd = small.tile([128, 1], F32, tag="n_sd", name=f"nsd_{ct}")
                nc.scalar.activation(out=sd, in_=var, func=AF.Sqrt, bias=eps_col)
                rstd = small.tile([128, 1], F32, tag="n_rstd", name=f"nrstd_{ct}")
                nc.vector.reciprocal(out=rstd, in_=sd)
                nc.vector.tensor_tensor(
                    out=A_cvec[:, ct:ct + 1], in0=gnw_sb[:, ct:ct + 1],
                    in1=rstd, op=ALU.mult,
                )
                p1 = small.tile([128, 1], F32, tag="n_p1", name=f"np1_{ct}")
                nc.vector.tensor_tensor(
                    out=p1, in0=A_cvec[:, ct:ct + 1], in1=msv, op=ALU.mult
                )
                nc.vector.tensor_tensor(out=p1, in0=p1, in1=EX, op=ALU.mult)
                nc.vector.tensor_tensor(
                    out=B_cvec[:, ct:ct + 1], in0=gnb_sb[:, ct:ct + 1],
                    in1=p1, op=ALU.subtract,
                )

            # broadcast A/Bc along partitions via a DRAM round trip (tiny)
            A_dram = dram.tile([C], F32)
            B_dram = dram.tile([C], F32)
            nc.gpsimd.dma_start(
                out=A_dram.rearrange("(t p) -> p t", p=128), in_=A_cvec
            )
            nc.gpsimd.dma_start(
                out=B_dram.rearrange("(t p) -> p t", p=128), in_=B_cvec
            )
            A_bcast = fin.tile([128, C], F32)
            B_bcast = fin.tile([128, C], F32)
            nc.gpsimd.dma_start(
                out=A_bcast, in_=A_dram.unsqueeze(0).partition_broadcast(128)
            )
            nc.gpsimd.dma_start(
                out=B_bcast, in_=B_dram.unsqueeze(0).partition_broadcast(128)
            )

            final = fin.tile([128, 4, C], F32)
            sqj = fin.tile([128, C], F32)
            for it in range(4):
                nc.vector.tensor_tensor(
                    out=final[:, it, :], in0=pre[:, it, :], in1=A_bcast,
                    op=ALU.mult,
                )
                nc.vector.tensor_tensor(
                    out=final[:, it, :], in0=final[:, it, :], in1=B_bcast,
                    op=ALU.add,
                )
                l2 = small.tile([128, 1], F32, tag="l2", name=f"l2_{it}")
                nc.scalar.activation(out=sqj, in_=final[:, it, :], func=AF.Square)
                nc.vector.tensor_reduce(
                    out=l2, in_=sqj, axis=mybir.AxisListType.X, op=ALU.add
                )
                sd2 = small.tile([128, 1], F32, tag="sd2", name=f"sd2_{it}")
                nc.scalar.activation(out=sd2, in_=l2, func=AF.Sqrt)
                rn = small.tile([128, 1], F32, tag="rn", name=f"rn_{it}")
                nc.vector.reciprocal(out=rn, in_=sd2)
                nc.vector.tensor_scalar(
                    out=final[:, it, :], in0=final[:, it, :],
                    scalar1=rn, scalar2=None, op0=ALU.mult,
                )
                nc.sync.dma_start(
                    out=out[it * 128:(it + 1) * 128, :], in_=final[:, it, :]
                )


def build_kernel(no_cc=False, n_cores=M, repeat=1, stage=3):
    nc = bacc.Bacc("TRN2", target_bir_lowering=False, debug=False,
                   num_devices=n_cores)

    xT_in = nc.dram_tensor("xT_bf", [N, B], BF16, kind="ExternalInput")
    xa_in = nc.dram_tensor("xa_bf", [N, B], BF16, kind="ExternalInput")
    Wq = nc.dram_tensor("Wq_bf", [N, C], BF16, kind="ExternalInput")
    Wk = nc.dram_tensor("Wk_bf", [N, C], BF16, kind="ExternalInput")
    Wv = nc.dram_tensor("Wv_bf", [N, C], BF16, kind="ExternalInput")
    Wskip = nc.dram_tensor("Wskip_bf", [N, C], BF16, kind="ExternalInput")
    bq = nc.dram_tensor("bq", [C], F32, kind="ExternalInput")
    bk = nc.dram_tensor("bk", [C], F32, kind="ExternalInput")
    bv = nc.dram_tensor("bv", [C], F32, kind="ExternalInput")
    bskip = nc.dram_tensor("bskip", [C], F32, kind="ExternalInput")
    we = nc.dram_tensor("we", [C], F32, kind="ExternalInput")
    gn_w = nc.dram_tensor("gn_w", [C], F32, kind="ExternalInput")
    gn_b = nc.dram_tensor("gn_b", [C], F32, kind="ExternalInput")
    gn_ms = nc.dram_tensor("gn_ms", [C], F32, kind="ExternalInput")
    out = nc.dram_tensor("out", [B, C], F32, kind="ExternalOutput")

    io = (xT_in, xa_in, Wq, Wk, Wv, Wskip, bq, bk, bv, bskip, we,
          gn_w, gn_b, gn_ms, out)
    groups = [list(range(n_cores))]

    with tile.TileContext(nc) as tc:
        for _rep in range(repeat):
            _emit_once(nc, tc, io, groups, no_cc, stage=stage)

    nc.finalize()
    return nc


_NC_CACHE = {}


def make_in_maps(inputs):
    """Host-side prep: slice per-core shards, pre-transpose x, cast to bf16."""
    x = np.ascontiguousarray(inputs["x"], dtype=np.float32)
    xT_bf = np.ascontiguousarray(x.T).astype(BF16_NP)   # [N, N]; col i = x[i,:]
    x_bf = x.astype(BF16_NP)
    w_bf = {
        k: np.ascontiguousarray(inputs[k], dtype=np.float32).astype(BF16_NP)
        for k in ("Wq", "Wk", "Wv", "Wskip")
    }
    f32v = {
        k: np.ascontiguousarray(inputs[k], dtype=np.float32)
        for k in ("bq", "bk", "bv", "bskip", "we", "gn_w", "gn_b", "gn_ms")
    }
    in_maps = []
    for m in range(M):
        I = slice(m * B, (m + 1) * B)
        im = {
            "xT_bf": np.ascontiguousarray(xT_bf[:, I]),
            "xa_bf": np.ascontiguousarray(x_bf[:, I]),
            "Wq_bf": w_bf["Wq"], "Wk_bf": w_bf["Wk"],
            "Wv_bf": w_bf["Wv"], "Wskip_bf": w_bf["Wskip"],
        }
        im.update(f32v)
        in_maps.append(im)
    return in_maps


def kernel(**inputs):
    if "nc" not in _NC_CACHE:
        _NC_CACHE["nc"] = build_kernel()
    nc = _NC_CACHE["nc"]
    in_maps = make_in_maps(inputs)
    res = run_bass_kernel_spmd(nc, in_maps, core_ids=list(range(M)))
    return np.concatenate([res.results[m]["out"] for m in range(M)], axis=0)


if __name__ == "__main__":
    data = np.load("/tmp/inputs.npz")
    out = kernel(**{k: data[k] for k in data.files})
    ref = np.load("/tmp/ref_out.npy")
    err = np.abs(out - ref)
    print("absmax", err.max(), "scale-rel", err.max() / np.abs(ref).max())
    print("rel2", np.linalg.norm(out - ref) / np.linalg.norm(ref))
